# revision 10
# baseline (speedup 1.0000x reference)
"""Causal attention (B=4, S=4096, D_IN=768, D_OUT=64) on 8 Trainium2 NeuronCores.

Sharding: core c handles batch b=c//2 and key-parity p=c%2 (the even or odd
128-wide key tiles of that batch). Every core computes, for ALL queries of its
batch, the unnormalized attention partials over its own key set:
    num[o, q] = sum_{k in own} exp(q.k/8) * V[k, o]
    den[q]    = sum_{k in own} exp(q.k/8)
The host sums the two partials per batch and normalizes: ctx = (num/den).T.
Causality is exact: key-tile work is skipped below the diagonal band and the
two boundary blocks are masked with host-provided mask tiles.

Host prep per core: x[b].T cast to bf16 with columns permuted to
[own key tiles | other key tiles] so the device program is identical across
cores (SPMD). Weights are pre-rearranged on host to the on-chip layout so
every DMA is contiguous, and x is fetched in 5 grouped DMAs (each covering
all 6 contraction chunks of 1-2 column blocks) because a DMA trigger costs
~790ns on the sync queue regardless of its descriptor count — few big
transfers keep the 2KB-line stream at full HBM rate from t~0.

All on-chip operands are bf16 (1 cyc/col on the PE at any moving width, half
the HBM/SBUF traffic of fp32); PSUM accumulation and the nd output stay fp32.
Query tiles run in the order 0,1,2,3,7,6,5,4; each tile's context matmuls are
drained interleaved between the NEXT tile's score pairs so the in-order PE
queue always holds ready work while the scalar engine catches up on exps
(which also keeps the PE at its ramped 2.4 GHz p-state). Causal-boundary
masking runs on the otherwise-idle gpsimd engine and the PSUM->SBUF output
copies on the scalar engine, so the DVE queue never blocks the PE.
"""
import numpy as np
import ml_dtypes

import concourse.bass as bass
import concourse.bacc as bacc
import concourse.tile as tile
from concourse import mybir
from concourse.bass_utils import run_bass_kernel_spmd

B, S, DI, DO = 4, 4096, 768, 64
NCORES = 8
NIC = DI // 128          # 6 contraction chunks
NKT = S // 128           # 32 global key tiles per batch
NOWN = NKT // 2          # 16 own key tiles per core
QT = 512                 # query tile width (one PSUM bank of fp32)
NQT = S // QT            # 8 query tiles
F32 = mybir.dt.float32
BF16 = mybir.dt.bfloat16
NPBF16 = ml_dtypes.bfloat16

T_ORDER = [0, 1, 2, 3, 7, 6, 5, 4]

_prog_cache = {}


def j0_of(T):
    """First diagonal-region packed key tile for permuted query tile T."""
    return 4 * T if T < 4 else 4 * (T - 4)


def build_program():
    """Build + compile the single SPMD Bass program (identical on all cores)."""
    nc = bacc.Bacc("TRN2", target_bir_lowering=False, debug=False)

    xT = nc.declare_dram_parameter("xT", [DI, S], BF16, isOutput=False)
    # weights already in on-chip layout: [partition, chunk*width]
    wkvr = nc.declare_dram_parameter("wkvr", [128, NIC * 128], BF16, isOutput=False)
    wqr = nc.declare_dram_parameter("wqr", [128, NIC * DO], BF16, isOutput=False)
    mboth = nc.declare_dram_parameter("mboth", [128, 256], BF16, isOutput=False)
    ident = nc.declare_dram_parameter("ident", [DO, DO], BF16, isOutput=False)
    nd = nc.declare_dram_parameter("nd", [DO + 1, S], F32, isOutput=True)

    with tile.TileContext(nc) as tc:
        with tc.tile_pool(name="consts", bufs=1) as consts, \
             tc.tile_pool(name="xpool", bufs=1) as xpool, \
             tc.tile_pool(name="qkv", bufs=1) as qkv, \
             tc.tile_pool(name="expp", bufs=10) as expp, \
             tc.tile_pool(name="ndst", bufs=2) as ndst, \
             tc.tile_pool(name="ps_proj", bufs=2, space="PSUM") as ps_proj, \
             tc.tile_pool(name="ps_sc2", bufs=2, space="PSUM") as ps_sc2, \
             tc.tile_pool(name="ps_ctx", bufs=2, space="PSUM") as ps_ctx:

            twkv = consts.tile([128, NIC * 128], BF16, tag="twkv", name="twkv")
            twq = consts.tile([128, NIC * DO], BF16, tag="twq", name="twq")
            tmsk = consts.tile([128, 256], BF16, tag="tmsk", name="tmsk")
            tid = consts.tile([DO, DO], BF16, tag="tid", name="tid")

            # x^T fetched in 5 grouped DMAs; group g holds all 6 contraction
            # chunks for its column block(s), ordered by first consumption.
            xg01a = xpool.tile([128, NIC // 2, 2 * QT], BF16, tag="xg01a", name="xg01a")
            xg01b = xpool.tile([128, NIC // 2, 2 * QT], BF16, tag="xg01b", name="xg01b")
            xg23 = xpool.tile([128, NIC, 2 * QT], BF16, tag="xg23", name="xg23")
            xg67 = xpool.tile([128, NIC, 2 * QT], BF16, tag="xg67", name="xg67")
            xg45 = xpool.tile([128, NIC, 2 * QT], BF16, tag="xg45", name="xg45")

            xr = xT.rearrange("(c p) w -> p c w", p=128)
            nc.sync.dma_start(out=twkv, in_=wkvr[:, :])
            nc.sync.dma_start(out=xg01a, in_=xr[:, 0:NIC // 2, 0:2 * QT])
            nc.sync.dma_start(out=twq, in_=wqr[:, :])
            nc.sync.dma_start(out=xg01b, in_=xr[:, NIC // 2:NIC, 0:2 * QT])
            nc.sync.dma_start(out=tmsk, in_=mboth[:, :])
            nc.sync.dma_start(out=tid, in_=ident[:, :])
            nc.sync.dma_start(out=xg23, in_=xr[:, :, 2 * QT:4 * QT])
            nc.sync.dma_start(out=xg67, in_=xr[:, :, 6 * QT:8 * QT])
            nc.sync.dma_start(out=xg45, in_=xr[:, :, 4 * QT:6 * QT])

            tmd = tmsk[:, 0:128]
            tmp = tmsk[:, 128:256]

            zsrc = consts.tile([DO, 1], F32, tag="zsrc", name="zsrc")
            nc.vector.memset(zsrc, 0.0)
            # Dummy exp to pull the ACT table load off the critical path.
            zexp = consts.tile([DO, 1], F32, tag="zexp", name="zexp")
            nc.scalar.activation(zexp, zsrc,
                                 mybir.ActivationFunctionType.Exp, scale=1.0)

            def xc(ic, cb):
                """[128, 512] view of column block cb, chunk ic."""
                if cb in (0, 1):
                    g = xg01a if ic < NIC // 2 else xg01b
                    return g[:, ic % (NIC // 2), cb * QT:(cb + 1) * QT]
                if cb in (2, 3):
                    return xg23[:, ic, (cb - 2) * QT:(cb - 1) * QT]
                if cb in (6, 7):
                    return xg67[:, ic, (cb - 6) * QT:(cb - 5) * QT]
                return xg45[:, ic, (cb - 4) * QT:(cb - 3) * QT]

            kts = [qkv.tile([DO, QT], BF16, tag=f"kt_{st}", name=f"kt_{st}") for st in range(4)]
            vts = [qkv.tile([DO, QT], BF16, tag=f"vt_{st}", name=f"vt_{st}") for st in range(4)]
            qts = [qkv.tile([DO, QT], BF16, tag=f"qt_{st}", name=f"qt_{st}") for st in range(NQT)]
            v1s = [qkv.tile([128, DO + 1], BF16, tag=f"v1_{j}", name=f"v1_{j}")
                   for j in range(NOWN)]

            def emit_pass1a(st):
                """[K^T | V^T] over own key column block st."""
                p1 = ps_proj.tile([128, QT], F32, tag="psproj", name="psproj")
                for ic in range(NIC):
                    nc.tensor.matmul(p1, twkv[:, ic * 128:(ic + 1) * 128],
                                     xc(ic, st),
                                     start=(ic == 0), stop=(ic == NIC - 1))
                nc.vector.tensor_copy(kts[st], p1[0:DO, :])
                nc.vector.tensor_copy(vts[st], p1[DO:128, :])

            def emit_pass1b(st):
                """V1 tiles for block st via DMA XBAR transpose (keeps the PE
                and its PSUM pools out of the V^T -> V1 path entirely)."""
                for j in range(4 * st, 4 * st + 4):
                    col = (j % 4) * 128
                    nc.sync.dma_start_transpose(v1s[j][:, 0:DO],
                                                vts[st][:, col:col + 128])
                    # ones column for the row-sum (denominator); tmd[:,127] == 1
                    nc.vector.tensor_copy(v1s[j][:, DO:DO + 1], tmd[:, 127:128])

            def emit_pass2(st):
                """Q^T over (permuted) query column block st."""
                p2 = ps_proj.tile([DO, QT], F32, tag="psproj", name="psproj")
                for ic in range(NIC):
                    nc.tensor.matmul(p2, twq[:, ic * DO:(ic + 1) * DO],
                                     xc(ic, st),
                                     start=(ic == 0), stop=(ic == NIC - 1))
                nc.vector.tensor_copy(qts[st], p2)

            exp_scale = float(1.0 / np.sqrt(DO))

            def emit_scores(T, j, sp_ap):
                """scores matmul for (T, j) into sp_ap ([128, w])."""
                r = j - j0_of(T)
                qlo = 128 * r if r > 0 else 0
                w = QT - qlo
                st, col = j // 4, (j % 4) * 128
                nc.tensor.matmul(sp_ap[:, 0:w], kts[st][:, col:col + 128],
                                 qts[T][:, qlo:QT], start=True, stop=True)
                return qlo, w

            class CtxDrain:
                """Phase B for a query tile, drained a few matmuls at a time
                between the next tile's scores pairs."""

                def __init__(self, T, ctx_args):
                    self.T = T
                    self.nk = j0_of(T) + 4
                    self.args = ctx_args
                    self.i = 0
                    self.ctxp = ps_ctx.tile([DO + 1, QT], F32, tag="ctxp",
                                            name="ctxp")

                def drain(self, n):
                    while self.i < len(self.args) and n > 0:
                        j, et_ap, qlo, w = self.args[self.i]
                        nc.tensor.matmul(self.ctxp[:, qlo:QT], v1s[j],
                                         et_ap[:, 0:w],
                                         start=(j == 0), stop=(j == self.nk - 1))
                        self.i += 1
                        n -= 1

                def finish(self):
                    self.drain(len(self.args))
                    ost = ndst.tile([DO + 1, QT], F32, tag="ost", name="ost")
                    nc.scalar.copy(ost, self.ctxp)
                    nc.sync.dma_start(out=nd[:, self.T * QT:(self.T + 1) * QT],
                                      in_=ost)

            emit_pass1a(0)
            emit_pass2(0)
            emit_pass1a(1)
            emit_pass1b(0)
            pass1a_done, pass1b_done = 2, 1
            pending = None  # CtxDrain from the previous iteration
            for ti, T in enumerate(T_ORDER):
                j0 = j0_of(T)
                nk = j0 + 4
                mask = tmd if T < 4 else tmp
                ctx_args = []   # (j, et_ap, qlo, w) consumed in phase B

                def emit_proj(ti=ti):
                    """Projections for the upcoming tiles, emitted after the
                    first score group so later groups hide the qts/kts cast."""
                    nonlocal pass1a_done, pass1b_done
                    if ti + 1 < NQT:
                        if pass1a_done < 4:
                            emit_pass1a(pass1a_done)
                            pass1a_done += 1
                        if pass1b_done < 4:
                            emit_pass1b(pass1b_done)
                            pass1b_done += 1
                        emit_pass2(T_ORDER[ti + 1])

                for p, j in enumerate(range(0, j0, 2)):
                    sp2 = ps_sc2.tile([128, 2 * QT], F32, tag="sp2", name="sp2")
                    et2 = expp.tile([128, 2 * QT], BF16, tag="et", name="et")
                    emit_scores(T, j, sp2[:, 0:QT])
                    emit_scores(T, j + 1, sp2[:, QT:2 * QT])
                    nc.scalar.activation(et2, sp2,
                                         mybir.ActivationFunctionType.Exp,
                                         scale=exp_scale)
                    ctx_args.append((j, et2[:, 0:QT], 0, QT))
                    ctx_args.append((j + 1, et2[:, QT:2 * QT], 0, QT))
                    if p == 0:
                        emit_proj()
                    if pending is not None:
                        pending.drain(2)
                # diagonal band: r=0 (w=512) + r=1 (w=384) share a 2-bank tile;
                # r=2 (w=256) + r=3 (w=128) share a 1-bank tile
                spb1 = ps_sc2.tile([128, 2 * QT], F32, tag="sp2", name="sp2")
                etb1 = expp.tile([128, 2 * QT], BF16, tag="et", name="et")
                emit_scores(T, j0, spb1[:, 0:QT])
                emit_scores(T, j0 + 1, spb1[:, QT:QT + 384])
                nc.scalar.activation(etb1[:, 0:QT + 384], spb1[:, 0:QT + 384],
                                     mybir.ActivationFunctionType.Exp,
                                     scale=exp_scale)
                nc.gpsimd.tensor_mul(etb1[:, 0:128], etb1[:, 0:128], mask)
                nc.vector.tensor_mul(etb1[:, QT:QT + 128], etb1[:, QT:QT + 128], mask)
                ctx_args.append((j0, etb1[:, 0:QT], 0, QT))
                ctx_args.append((j0 + 1, etb1[:, QT:QT + 384], 128, 384))
                if j0 == 0:
                    emit_proj()
                if pending is not None:
                    pending.drain(2)
                spb2 = ps_proj.tile([128, QT], F32, tag="psproj", name="psproj")
                etb2 = expp.tile([128, 2 * QT], BF16, tag="et", name="et")
                emit_scores(T, j0 + 2, spb2[:, 0:256])
                emit_scores(T, j0 + 3, spb2[:, 256:384])
                nc.scalar.activation(etb2[:, 0:384], spb2[:, 0:384],
                                     mybir.ActivationFunctionType.Exp,
                                     scale=exp_scale)
                nc.gpsimd.tensor_mul(etb2[:, 0:128], etb2[:, 0:128], mask)
                nc.vector.tensor_mul(etb2[:, 256:384], etb2[:, 256:384], mask)
                ctx_args.append((j0 + 2, etb2[:, 0:256], 256, 256))
                ctx_args.append((j0 + 3, etb2[:, 256:384], 384, 128))

                if pending is not None:
                    pending.finish()
                pending = CtxDrain(T, ctx_args)
            pending.finish()

    nc.compile()
    return nc


def get_program():
    if "nc" not in _prog_cache:
        _prog_cache["nc"] = build_program()
    return _prog_cache["nc"]


def core_perm(parity):
    """Permuted-to-global column index map: own key tiles first, then other."""
    own = [g for g in range(NKT) if g % 2 == parity]
    other = [g for g in range(NKT) if g % 2 != parity]
    return np.concatenate([np.arange(g * 128, (g + 1) * 128) for g in own + other])


def make_in_maps(x, Wq, Wk, Wv):
    x = np.asarray(x, dtype=np.float32)
    Wq = np.asarray(Wq, dtype=np.float32)
    Wk = np.asarray(Wk, dtype=np.float32)
    Wv = np.asarray(Wv, dtype=np.float32)
    wkv = np.concatenate([Wk, Wv], axis=1)
    # on-chip layout: [partition p, chunk c, width] flattened
    wkvr = np.ascontiguousarray(
        wkv.reshape(NIC, 128, 128).transpose(1, 0, 2).reshape(128, NIC * 128)
    ).astype(NPBF16)
    wqr = np.ascontiguousarray(
        Wq.reshape(NIC, 128, DO).transpose(1, 0, 2).reshape(128, NIC * DO)
    ).astype(NPBF16)
    mdiag = np.triu(np.ones((128, 128), dtype=np.float32))
    ident = np.eye(DO, dtype=np.float32).astype(NPBF16)
    in_maps = []
    perms = []
    for c in range(NCORES):
        b, par = c // 2, c % 2
        perm = core_perm(par)
        perms.append(perm)
        xTp = np.ascontiguousarray(x[b].T[:, perm].astype(NPBF16))
        mpcol = np.full((128, 128), 1.0 - par, dtype=np.float32)
        mb = np.concatenate([mdiag, mpcol], axis=1).astype(NPBF16)
        in_maps.append({
            "xT": xTp, "wkvr": wkvr, "wqr": wqr,
            "mboth": mb, "ident": ident,
        })
    return in_maps, perms


def combine(results, perms):
    out = np.empty((B, S, DO), dtype=np.float32)
    for b in range(B):
        num = np.zeros((DO, S), dtype=np.float64)
        den = np.zeros((S,), dtype=np.float64)
        for c in (2 * b, 2 * b + 1):
            nd_c = results[c]["nd"].astype(np.float64)
            inv = np.empty(S, dtype=np.int64)
            inv[perms[c]] = np.arange(S)
            nd_g = nd_c[:, inv]
            num += nd_g[:DO]
            den += nd_g[DO]
        out[b] = (num / den).T.astype(np.float32)
    return out


def kernel(x, Wq, Wk, Wv):
    nc = get_program()
    in_maps, perms = make_in_maps(x, Wq, Wk, Wv)
    res = run_bass_kernel_spmd(nc, in_maps, list(range(NCORES)))
    return combine(res.results, perms)
